# revision 1
# baseline (speedup 1.0000x reference)
"""EMAttention2d (vq_codebook) Trainium2 kernel.

Data parallel over batch: 16 images -> 8 cores x 2 images. Single kernel
launch per core; BN batch stats cross-core reduced with a tiny AllReduce.

Math (validated vs reference, fp32):
  per image, X = x[b] reshaped (C, N):
    mu_0 = mu
    repeat 3x:
      mutT = mu^T Ws            (K, C)      [stem folded into codebook]
      mub  = mu^T bs            (K,)
      A^T  = mutT X + mub       (K, N)
      E^T  = exp(A^T)                      [no max-sub: logits bounded]
      E    = transpose(E^T)     (N, K)
      s_n  = sum_k E            (N,)
      z    = E / s_n            (N, K)     [proper softmax]
      G    = z^T X^T            (K, C)
      s_k  = z^T 1              (K,)
      muRT = G Ws^T + s_k bs^T  (K, C)
      muT  = muRT / ||row||_2              [zn / 1e-6 normalizers cancel]
    y2   = mu3 z3^T             (C, N)
    G2   = Hw relu(y2)          (C, N)     [head bias drops out of BN]
  BN over batch of G2 (AllReduce of per-channel sum/sumsq), then
    out = relu(G2 * a + b2 + x),  a = gamma*rstd, b2 = beta - mean*a

SBUF phasing (pools are stack-scoped):
  L1 (whole kernel): consts, z (2 imgs), muT (2 imgs), small staging, psum
  L2 (transpose+EM of both imgs): X, X^T, exp staging   -- 134 KB/part
  L3 (y2/head/BN/final, opened after L2 closes): head weights, zT, ry2,
     h1 (img1 pre-BN acts, bf16, SBUF), h0 spill staging, final staging
"""

import sys

for _p in ("/opt/trn_rl_repo",):
    if _p not in sys.path:
        sys.path.insert(0, _p)

import numpy as np

B, C, N, K = 16, 512, 4096, 64
NCORES = 8
BPC = B // NCORES  # images per core
P = 128
OC = C // P   # 4 chunks of channels
NT = N // P   # 32 pixel tiles
NCH = N // 512  # 8 pixel chunks of 512
BN_EPS = 1e-5
NUM_ITER = 3

_cache = {}


def _build_nc(n_devices=NCORES, use_collective=True):
    import concourse.bass as bass
    import concourse.mybir as mybir
    import concourse.tile as tile
    from concourse.masks import make_identity
    from contextlib import ExitStack

    dt = mybir.dt
    f32 = dt.float32
    f32r = dt.float32r
    bf16 = dt.bfloat16
    AF = mybir.ActivationFunctionType
    ALU = mybir.AluOpType
    AX = mybir.AxisListType

    nc = bass.Bass("TRN2", target_bir_lowering=False, debug=False,
                   num_devices=n_devices)

    x_d = nc.dram_tensor("x", [BPC, C, N], f32, kind="ExternalInput").ap()
    mu_d = nc.dram_tensor("mu", [C, K], f32, kind="ExternalInput").ap()
    ws_d = nc.dram_tensor("ws", [C, C], f32, kind="ExternalInput").ap()
    wst_d = nc.dram_tensor("wst", [C, C], f32, kind="ExternalInput").ap()
    hwt_d = nc.dram_tensor("hwt", [C, C], f32, kind="ExternalInput").ap()
    bs_d = nc.dram_tensor("bs", [C, 2], f32, kind="ExternalInput").ap()
    gm_d = nc.dram_tensor("gm", [C], f32, kind="ExternalInput").ap()
    onec_d = nc.dram_tensor("onec", [P], f32, kind="ExternalInput").ap()
    bt_d = nc.dram_tensor("bt", [C], f32, kind="ExternalInput").ap()
    out_d = nc.dram_tensor("out", [BPC, C, N], f32, kind="ExternalOutput").ap()
    h0_d = nc.dram_tensor("h0spill", [C, N], bf16).ap()
    st_in_d = nc.dram_tensor("stats_in", [P, 2 * OC], f32).ap()
    st_out_d = nc.dram_tensor("stats_out", [P, 2 * OC], f32,
                              addr_space="Shared").ap()

    with tile.TileContext(nc) as tc, ExitStack() as ctx:
        consts = ctx.enter_context(tc.tile_pool(name="consts", bufs=1))
        zpool = ctx.enter_context(tc.tile_pool(name="zpool", bufs=2))
        mutp = ctx.enter_context(tc.tile_pool(name="mutp", bufs=4))
        munp = ctx.enter_context(tc.tile_pool(name="munp", bufs=2))
        smalls = ctx.enter_context(tc.tile_pool(name="smalls", bufs=2))
        statp = ctx.enter_context(tc.tile_pool(name="statp", bufs=1))

        # ---- constants ----
        id_sb = consts.tile([P, P], f32)
        make_identity(nc, id_sb[:])
        ws_sb = consts.tile([P, OC, C], f32r)    # Ws rows  (o_part, oc, ci)
        nc.sync.dma_start(ws_sb[:], ws_d.rearrange("(t p) c -> p t c", p=P).bitcast(f32r))
        wst_sb = consts.tile([P, OC, C], f32r)   # Ws^T rows (ci_part, cc, o)
        nc.sync.dma_start(wst_sb[:], wst_d.rearrange("(t p) c -> p t c", p=P).bitcast(f32r))
        mu0_sb = consts.tile([P, OC, K], f32r)
        nc.sync.dma_start(mu0_sb[:], mu_d.rearrange("(t p) k -> p t k", p=P).bitcast(f32r))
        b_sb = consts.tile([P, OC, 2], f32r)
        nc.sync.dma_start(b_sb[:],
                          bs_d.rearrange("(t p) two -> p t two", p=P)
                          .bitcast(f32r))
        bs_row = consts.tile([1, C], f32r)
        nc.sync.dma_start(bs_row[:], bs_d[:, 0][None, :].bitcast(f32r))
        gm_sb = consts.tile([P, OC], f32)
        nc.sync.dma_start(gm_sb[:], gm_d.rearrange("(t p) -> p t", p=P))
        bt_sb = consts.tile([P, OC], f32)
        nc.sync.dma_start(bt_sb[:], bt_d.rearrange("(t p) -> p t", p=P))
        ones_col = consts.tile([P, 1], f32r)
        nc.sync.dma_start(ones_col[:], onec_d[:, None].bitcast(f32r))
        eps_sb = consts.tile([P, 1], f32)
        nc.vector.memset(eps_sb[:], BN_EPS)
        idr = id_sb[:]  # fp32 transposes (f32r transpose trips walrus sync-wait limit)

        sum_acc = statp.tile([P, OC, BPC * NCH], f32)
        sq_acc = statp.tile([P, OC, BPC * NCH], f32)

        z_of = [None] * BPC
        muT_of = [None] * BPC

        # ================= L2: transpose + EM for both images ============
        with ExitStack() as l2:
            xpool = l2.enter_context(tc.tile_pool(name="xpool", bufs=16))
            xtpool = l2.enter_context(tc.tile_pool(name="xtpool", bufs=32))
            etpool = l2.enter_context(tc.tile_pool(name="etpool", bufs=3))
            psum2 = l2.enter_context(tc.tile_pool(name="psum2", bufs=1,
                                                  space="PSUM"))

            def ps(nm, bufs_tag=None):
                return psum2.tile([P, 512], f32, tag=bufs_tag or nm, name=nm)

            for b in range(BPC):
                # X as 16 quarter tiles (cc, q) so transposes start after
                # the first 2 MiB of the image load
                xh = {}
                for hf in range(4):
                    for cc in range(OC):
                        xc = xpool.tile([P, N // 4], f32r, tag="X",
                                        name=f"x{b}_{cc}_{hf}")
                        nc.sync.dma_start(
                            xc[:],
                            x_d[b, cc * P:(cc + 1) * P,
                                hf * (N // 4):(hf + 1) * (N // 4)]
                            .bitcast(f32r))
                        xh[(cc, hf)] = xc

                def xs(cc, col, width):
                    hf = col // (N // 4)
                    off = col - hf * (N // 4)
                    return xh[(cc, hf)][:, off:off + width]
                xt_sb = []
                for tt in range(NT):
                    xt = xtpool.tile([P, C], f32r, tag="xT", name=f"xt{b}_{tt}")
                    pst = ps("xtr", "xtr%d" % (tt % 2))
                    for cc in range(OC):
                        nc.tensor.transpose(
                            pst[:, cc * P:(cc + 1) * P],
                            xs(cc, tt * P, P).bitcast(f32),
                            idr)
                    if tt % 2 == 0:
                        nc.vector.tensor_copy(xt[:], pst[:])
                    else:
                        nc.scalar.copy(xt[:], pst[:])
                    xt_sb.append(xt)

                # ---- EM iterations ----
                mu_nat = mu0_sb  # (P, OC, K) natural layout of current mu
                z_sb = zpool.tile([P, NT, K], f32r, tag="z", name=f"z{b}")
                z_of[b] = z_sb
                for it in range(NUM_ITER):
                    # mu~^T = mu^T Ws (K, C); transpose to (ci, k) chunks
                    mutT_ps = ps("mm")
                    for oc in range(OC):
                        nc.tensor.matmul(mutT_ps[:K, :].bitcast(f32),
                                         mu_nat[:, oc, :],
                                         ws_sb[:, oc, :],
                                         start=(oc == 0), stop=(oc == OC - 1))
                    mutT_sb = smalls.tile([K, C], f32r, tag="kc")
                    nc.vector.tensor_copy(mutT_sb[:], mutT_ps[:K, :])
                    mut_ps = ps("mm2")
                    for cc in range(OC):
                        nc.tensor.transpose(
                            mut_ps[:, cc * K:(cc + 1) * K],
                            mutT_sb[:, cc * P:(cc + 1) * P].bitcast(f32),
                            idr[:K, :K])
                    mut_sb = smalls.tile([P, OC, K], f32r, tag="mut")
                    nc.vector.tensor_copy(mut_sb[:], mut_ps[:, :OC * K])

                    # mub = mu^T bs  (K, 1)
                    mub_ps = ps("mm2")
                    for oc in range(OC):
                        nc.tensor.matmul(mub_ps[:K, :2].bitcast(f32),
                                         mu_nat[:, oc, :],
                                         b_sb[:, oc, :],
                                         start=(oc == 0), stop=(oc == OC - 1))
                    mub_sb = smalls.tile([K, 1], f32, tag="mub")
                    nc.vector.tensor_copy(mub_sb[:], mub_ps[:K, :1])

                    # A^T chunks -> exp -> transpose -> z
                    def z_block(et, ch):
                        e_ps = ps("Etr", "Etr%d" % (ch % 2))
                        e3 = e_ps[:, :4 * K].rearrange("p (j k) -> p j k",
                                                       k=K)
                        for j in range(4):
                            nc.tensor.transpose(
                                e3[:, j, :],
                                et[:, j * P:(j + 1) * P].bitcast(f32),
                                idr[:K, :K])
                        s4 = smalls.tile([P, 4], f32, tag="s4")
                        nc.vector.tensor_reduce(s4[:], e3[:], axis=AX.X,
                                                op=ALU.add)
                        nc.vector.reciprocal(s4[:], s4[:])
                        nc.vector.tensor_tensor(
                            z_sb[:, ch * 4:(ch + 1) * 4, :], e3[:],
                            s4[:, :, None].to_broadcast((P, 4, K)), ALU.mult)

                    pend = None
                    for ch in range(NCH):
                        a_ps = ps("A", "A%d" % (ch % 2))
                        for cc in range(OC):
                            nc.tensor.matmul(
                                a_ps[:K, :].bitcast(f32),
                                mut_sb[:, cc, :],
                                xs(cc, ch * 512, 512),
                                start=(cc == 0), stop=(cc == OC - 1))
                        et = etpool.tile([K, 512], f32r, tag="ET")
                        nc.scalar.activation(et[:], a_ps[:K, :], AF.Exp,
                                             bias=mub_sb[:], scale=1.0)
                        if pend is not None:
                            z_block(*pend)
                        pend = (et, ch)
                    z_block(*pend)

                    # G = z^T X^T ; s_k = z^T 1  (accumulate over tiles)
                    G_ps = ps("G", "xtr0")
                    sk_ps = ps("sk", "mm2")
                    for tt in range(NT):
                        nc.tensor.matmul(G_ps[:K, :].bitcast(f32),
                                         z_sb[:, tt, :],
                                         xt_sb[tt][:],
                                         start=(tt == 0), stop=(tt == NT - 1))
                    for tq in range(NT // 4):
                        nc.tensor.matmul(
                            sk_ps[:1, :4 * K].bitcast(f32),
                            ones_col[:],
                            z_sb[:, 4 * tq:4 * (tq + 1), :],
                            start=(tq == 0), stop=(tq == NT // 4 - 1))
                    g_sb = smalls.tile([K, C], f32r, tag="kc")
                    nc.vector.tensor_copy(g_sb[:], G_ps[:K, :])
                    sk_sb = smalls.tile([1, K], f32r, tag="sk")
                    with nc.allow_low_precision(
                            reason="f32r is 32-bit; rounding to f32r grid"):
                        nc.vector.tensor_reduce(
                            sk_sb[:],
                            sk_ps[:1, :4 * K].rearrange("p (f k) -> p k f",
                                                        k=K),
                            axis=AX.X, op=ALU.add)
                    gt_ps = ps("mm")
                    g3 = gt_ps[:, :OC * K].rearrange("p (j k) -> p j k", k=K)
                    for cc in range(OC):
                        nc.tensor.transpose(
                            g3[:, cc, :],
                            g_sb[:, cc * P:(cc + 1) * P].bitcast(f32),
                            idr[:K, :K])
                    gt_sb = smalls.tile([P, OC, K], f32r, tag="mut")
                    nc.vector.tensor_copy(gt_sb[:], g3[:])

                    # muR^T = G Ws^T + s_k bs^T  (K, C)
                    mur_ps = ps("mm2")
                    for cc in range(OC):
                        nc.tensor.matmul(mur_ps[:K, :].bitcast(f32),
                                         gt_sb[:, cc, :],
                                         wst_sb[:, cc, :],
                                         start=(cc == 0), stop=False)
                    nc.tensor.matmul(mur_ps[:K, :].bitcast(f32),
                                     sk_sb[:],
                                     bs_row[:],
                                     start=False, stop=True)
                    # muT = muR^T / ||row||_2
                    sq_sb = smalls.tile([K, C], f32, tag="kc")
                    nc.scalar.square(sq_sb[:], mur_ps[:K, :])
                    nrm = smalls.tile([K, 1], f32, tag="nrm")
                    nc.vector.tensor_reduce(nrm[:], sq_sb[:], axis=AX.X,
                                            op=ALU.add)
                    nc.scalar.activation(nrm[:], nrm[:], AF.Ln)
                    nc.scalar.activation(nrm[:], nrm[:], AF.Exp, scale=-0.5)
                    muT_sb = mutp.tile([K, C], f32r, tag="muT",
                                       name=f"muT{b}_{it}")
                    nc.vector.tensor_scalar(muT_sb[:], mur_ps[:K, :], nrm[:],
                                            None, ALU.mult)
                    if it < NUM_ITER - 1:
                        mun_ps = ps("mm")
                        m3 = mun_ps[:, :OC * K].rearrange(
                            "p (j k) -> p j k", k=K)
                        for ot in range(OC):
                            nc.tensor.transpose(
                                m3[:, ot, :],
                                muT_sb[:, ot * P:(ot + 1) * P].bitcast(f32),
                                idr[:K, :K])
                        mu_nat = munp.tile([P, OC, K], f32r, tag="munat")
                        nc.vector.tensor_copy(mu_nat[:], m3[:])
                muT_of[b] = muT_sb

        # ================= L3: y2 / head / BN / final ====================
        with ExitStack() as l3:
            hwp = l3.enter_context(tc.tile_pool(name="hwp", bufs=1))
            psum3 = l3.enter_context(tc.tile_pool(name="psum3", bufs=1,
                                                  space="PSUM"))

            def ps(nm, bufs_tag=None):
                return psum3.tile([P, 512], f32, tag=bufs_tag or nm, name=nm)
            ztpool = l3.enter_context(tc.tile_pool(name="ztpool", bufs=2))
            ry2pool = l3.enter_context(tc.tile_pool(name="ry2pool", bufs=2))
            hstage = l3.enter_context(tc.tile_pool(name="hstage", bufs=4))
            hbig = l3.enter_context(tc.tile_pool(name="hbig", bufs=1))
            fstage = l3.enter_context(tc.tile_pool(name="fstage", bufs=3))
            fload = l3.enter_context(tc.tile_pool(name="fload", bufs=8))

            hwt_sb = hwp.tile([P, OC, C], f32r)  # Hw^T rows (o_part, oc, o2)
            nc.sync.dma_start(hwt_sb[:],
                              hwt_d.rearrange("(t p) c -> p t c", p=P)
                              .bitcast(f32r))
            h1_sb = hbig.tile([P, OC, N], bf16)
            h0_sb = hbig.tile([P, OC, N], bf16)
            h_of = [h0_sb, h1_sb]

            for b in range(BPC):
                z_sb = z_of[b]
                muT_sb = muT_of[b]
                for ch in range(NCH):
                    zt_ps = ps("ztr", "ztr%d" % (ch % 2))
                    z4 = zt_ps[:, :4 * P].rearrange("p (j q) -> p j q", q=P)
                    for j in range(4):
                        nc.tensor.transpose(
                            z4[:K, j, :],
                            z_sb[:, ch * 4 + j, :].bitcast(f32),
                            idr)
                    zt_sb = ztpool.tile([K, 512], f32r, tag="zT")
                    nc.vector.tensor_copy(zt_sb[:], zt_ps[:K, :4 * P])
                    ry2 = ry2pool.tile([P, OC, 512], f32r, tag="ry2")
                    for ot in range(OC):
                        y2_ps = ps("y2", "y2%d" % (ot % 2))
                        nc.tensor.matmul(y2_ps[:].bitcast(f32),
                                         muT_sb[:, ot * P:(ot + 1) * P],
                                         zt_sb[:],
                                         start=True, stop=True)
                        nc.vector.tensor_scalar(ry2[:, ot, :], y2_ps[:],
                                                0.0, None, ALU.max)
                    for o2 in range(OC):
                        h_ps = ps("h", "h%d" % (o2 % 2))
                        for oc in range(OC):
                            nc.tensor.matmul(
                                h_ps[:].bitcast(f32),
                                hwt_sb[:, oc, o2 * P:(o2 + 1) * P],
                                ry2[:, oc, :],
                                start=(oc == 0), stop=(oc == OC - 1))
                        acol = b * NCH + ch
                        dap = h_of[b][:, o2, ch * 512:(ch + 1) * 512]
                        if o2 % 2 == 0:
                            nc.vector.tensor_scalar(
                                dap, h_ps[:], 0.0, 0.0, ALU.add, ALU.add,
                                accum_out=sum_acc[:, o2, acol:acol + 1])
                        else:
                            nc.scalar.activation(
                                dap, h_ps[:], AF.Copy,
                                accum_out=sum_acc[:, o2, acol:acol + 1])
                        junk = hstage.tile([P, 512], bf16, tag="junk")
                        nc.scalar.activation(
                            junk[:], h_ps[:], AF.Square,
                            accum_out=sq_acc[:, o2, acol:acol + 1])

            # ---- prefetch first final-pass chunks (no dep on stats) ----
            FCH = N // 1024
            forder = [(b, o2, fc) for b in range(BPC)
                      for o2 in range(OC) for fc in range(FCH)]
            fql = []
            for (b, o2, fc) in forder[:8]:
                xr = fload.tile([P, 1024], f32, tag="xr")
                nc.sync.dma_start(
                    xr[:], x_d[b, o2 * P:(o2 + 1) * P,
                               fc * 1024:(fc + 1) * 1024])
                fql.append(xr)

            # ---- BN stats: aggregate, AllReduce, affine coefficients ----
            pack = statp.tile([P, 2 * OC], f32)
            packv = pack[:].rearrange("p (o two) -> p o two", two=2)
            nc.vector.tensor_reduce(packv[:, :, 0:1], sum_acc[:], axis=AX.X,
                                    op=ALU.add)
            nc.vector.tensor_reduce(packv[:, :, 1:2], sq_acc[:], axis=AX.X,
                                    op=ALU.add)
            nc.sync.dma_start(st_in_d[:], pack[:])
            if use_collective:
                nc.gpsimd.collective_compute(
                    "AllReduce", ALU.add,
                    replica_groups=[list(range(n_devices))],
                    ins=[st_in_d[:]],
                    outs=[st_out_d[:]],
                )
            else:
                nc.sync.dma_start(st_out_d[:], st_in_d[:])
            red = statp.tile([P, 2 * OC], f32)
            nc.sync.dma_start(red[:], st_out_d[:])
            a_sb = statp.tile([P, OC], f32)
            b2_sb = statp.tile([P, OC], f32)
            inv_nb = 1.0 / float(B * N)
            for o2 in range(OC):
                mean = smalls.tile([P, 1], f32, tag="mean")
                nc.vector.tensor_scalar(mean[:], red[:, 2 * o2:2 * o2 + 1],
                                        inv_nb, None, ALU.mult)
                var = smalls.tile([P, 1], f32, tag="var")
                nc.vector.tensor_scalar(var[:],
                                        red[:, 2 * o2 + 1:2 * o2 + 2],
                                        inv_nb, None, ALU.mult)
                msq = smalls.tile([P, 1], f32, tag="msq")
                nc.vector.tensor_tensor(msq[:], mean[:], mean[:], ALU.mult)
                nc.vector.tensor_tensor(var[:], var[:], msq[:], ALU.subtract)
                # rstd = exp(-0.5*ln(var+eps))
                nc.scalar.activation(var[:], var[:], AF.Ln, bias=eps_sb[:])
                nc.scalar.activation(var[:], var[:], AF.Exp, scale=-0.5)
                nc.vector.tensor_tensor(a_sb[:, o2:o2 + 1],
                                        gm_sb[:, o2:o2 + 1], var[:],
                                        ALU.mult)
                nc.vector.tensor_tensor(msq[:], mean[:], a_sb[:, o2:o2 + 1],
                                        ALU.mult)
                nc.vector.tensor_tensor(b2_sb[:, o2:o2 + 1],
                                        bt_sb[:, o2:o2 + 1], msq[:],
                                        ALU.subtract)

            # ---- final: out = relu(h*a + b2 + x) ----
            for fi, (b, o2, fc) in enumerate(forder):
                if fi < len(fql):
                    xr = fql[fi]
                else:
                    xr = fload.tile([P, 1024], f32, tag="xr")
                    nc.sync.dma_start(
                        xr[:], x_d[b, o2 * P:(o2 + 1) * P,
                                   fc * 1024:(fc + 1) * 1024])
                hap = h_of[b][:, o2, fc * 1024:(fc + 1) * 1024]
                t1 = fstage.tile([P, 1024], f32, tag="t1")
                nc.vector.scalar_tensor_tensor(
                    t1[:], hap, a_sb[:, o2:o2 + 1], xr[:],
                    ALU.mult, ALU.add)
                otile = fstage.tile([P, 1024], f32, tag="ot")
                nc.vector.tensor_scalar(otile[:], t1[:],
                                        b2_sb[:, o2:o2 + 1], 0.0,
                                        ALU.add, ALU.max)
                nc.sync.dma_start(
                    out_d[b, o2 * P:(o2 + 1) * P,
                          fc * 1024:(fc + 1) * 1024], otile[:])

    _hoist_extra_waits(nc)
    return nc


_ENGINE_SEM_PREFIX = {
    "EngineType.PE": "PE_",
    "EngineType.Activation": "Activation_",
    "EngineType.DVE": "DVE_",
    "EngineType.Pool": "Pool_",
    "EngineType.SP": "SP_",
}


def _hoist_extra_waits(nc):
    """This walrus build rejects compute-engine instructions carrying more
    than one sync wait. Engine queues are strict FIFO, so (a) an
    instruction waiting on its own engine's semaphore is always already
    satisfied -> drop it; (b) any extra waits can be hoisted onto NoOp
    instructions injected just before, one wait each -- identical
    semantics."""
    import concourse.mybir as mybir
    nid = 0
    for blk in nc.m.functions[0].blocks:
        out = []
        changed = False
        for i in blk.instructions:
            si = getattr(i, "sync_info", None)
            eng = str(getattr(i, "engine", None))
            waits = list(si.on_wait) if si and si.on_wait else []
            if len(waits) > 1 and eng in _ENGINE_SEM_PREFIX:
                selfp = _ENGINE_SEM_PREFIX[eng]
                waits = [w for w in waits if not w.ant_name.startswith(selfp)]
                for w in waits[:-1]:
                    nid += 1
                    out.append(mybir.InstNoOp(
                        name=f"I-waitnop-{nid}",
                        engine=i.engine,
                        sync_info=mybir.SyncInfo(on_wait=[w], on_update=[]),
                        bass_nofuse=True,
                    ))
                i.sync_info = mybir.SyncInfo(
                    on_wait=waits[-1:], on_update=list(si.on_update or []))
                changed = True
            out.append(i)
        if changed:
            blk.instructions = out


def get_nc():
    if "nc" not in _cache:
        _cache["nc"] = _build_nc()
    return _cache["nc"]


def run(inputs_by_core, trace=False):
    from concourse.bass_utils import run_bass_kernel_spmd
    nc = get_nc()
    return run_bass_kernel_spmd(nc, inputs_by_core, list(range(NCORES)),
                                trace=trace)


def make_in_maps(x, mu, stem_w, stem_b, head_w, head_b, bn_gamma, bn_beta):
    x = np.ascontiguousarray(np.asarray(x, np.float32)).reshape(B, C, N)
    common = {
        "mu": np.ascontiguousarray(np.asarray(mu, np.float32)),
        "ws": np.ascontiguousarray(np.asarray(stem_w, np.float32)),
        "wst": np.ascontiguousarray(np.asarray(stem_w, np.float32).T),
        "hwt": np.ascontiguousarray(np.asarray(head_w, np.float32).T),
        "bs": np.ascontiguousarray(
            np.stack([np.asarray(stem_b, np.float32),
                      np.zeros(C, np.float32)], axis=1)),
        "gm": np.ascontiguousarray(np.asarray(bn_gamma, np.float32)),
        "onec": np.ones(128, np.float32),
        "bt": np.ascontiguousarray(np.asarray(bn_beta, np.float32)),
    }
    return [
        {"x": np.ascontiguousarray(x[i * BPC:(i + 1) * BPC]), **common}
        for i in range(NCORES)
    ]


def kernel(x, mu, stem_w, stem_b, head_w, head_b, bn_gamma, bn_beta):
    in_maps = make_in_maps(x, mu, stem_w, stem_b, head_w, head_b,
                           bn_gamma, bn_beta)
    res = run(in_maps, trace=False)
    out = np.concatenate([res.results[i]["out"] for i in range(NCORES)],
                         axis=0)
    return out.reshape(B, C, 64, 64).astype(np.float32)



# revision 16
# speedup vs baseline: 1.3629x; 1.3629x over previous
"""EMAttention2d (vq_codebook) Trainium2 kernel, v2.

Data parallel over batch: 16 images -> 8 cores x 2 images. BN batch stats
cross-core reduced with a tiny AllReduce.

Key layout change vs v1: the EM loop works in pixel-partition layout so
softmax needs no transposes, and all big matmuls keep a full 128-row
output partition with bf16 moving operands (1 cycle/row on PE):

  per image (X = x[b] as (C,N), host also supplies X^T; both bf16):
    mu~ = Ws^T mu          (C,K)  [stem folded into codebook; host for it0]
    mub = mu^T bs          (K,)   [host for it0]
    repeat 3x:
      A[n,k]  = X^T mu~ + 1 (x) mub    - 4 chunk matmuls + rank-1 per tile
      E       = exp(A)                 (N,K) bf16, pixel-partition
      z       = E / rowsum(E)          softmax over free dim k
      Gx[c,k] = sum_n X[c,n] z[n,k]    - lhsT = X^T tiles, rhs = z tiles
      sk      = 1^T z
      muR     = Ws Gx + bs (x) sk      (C,K) natural layout
      mu      = muR / ||col||_2        - norm via ones^T muR^2 matmuls
    y2   = mu z^T   (relu) -> head Hw  - z^T via 1c/r bf16 PE transposes
  BN over batch (AllReduce of per-channel sum/sumsq), then
    out = relu(h*a + b2 + x),  a = gamma*rstd, b2 = beta - mean*a

x stays resident in SBUF (bf16) so the final pass reloads nothing.
"""

import sys

for _p in ("/opt/trn_rl_repo",):
    if _p not in sys.path:
        sys.path.insert(0, _p)

import numpy as np

B, C, N, K = 16, 512, 4096, 64
NCORES = 8
BPC = B // NCORES  # images per core
P = 128
OC = C // P   # 4 chunks of channels
NT = N // P   # 32 pixel tiles
NBK = 4       # A-banks per EM iteration
TPB = NT // NBK  # pixel tiles per bank (8)
FCH = N // 1024  # final-pass chunks per (img, o2)
BN_EPS = 1e-5
NUM_ITER = 3

_cache = {}


def _build_nc(n_devices=NCORES, use_collective=True, debug_dumps=False):
    import concourse.bass as bass
    import concourse.mybir as mybir
    import concourse.tile as tile
    from concourse.masks import make_identity
    from contextlib import ExitStack

    dt = mybir.dt
    f32 = dt.float32
    bf16 = dt.float16  # fp16 storage: 8x finer rounding than bf16, same engine rates
    bfr = dt.bfloat16
    AF = mybir.ActivationFunctionType
    ALU = mybir.AluOpType
    AX = mybir.AxisListType

    nc = bass.Bass("TRN2", target_bir_lowering=False, debug=False,
                   num_devices=n_devices)

    xb_d = nc.dram_tensor("xb", [BPC, P, OC, N], bf16, kind="ExternalInput").ap()
    xt_d = nc.dram_tensor("xt", [BPC, P, NT, C], bf16, kind="ExternalInput").ap()
    ws_d = nc.dram_tensor("ws", [P, OC, C], bf16, kind="ExternalInput").ap()
    wst_d = nc.dram_tensor("wst", [P, OC, C], bf16, kind="ExternalInput").ap()
    hwt_d = nc.dram_tensor("hwt", [P, OC, C], bf16, kind="ExternalInput").ap()
    mut0_d = nc.dram_tensor("mut0", [P, OC, K], bf16, kind="ExternalInput").ap()
    mub0_d = nc.dram_tensor("mub0", [1, K], bf16, kind="ExternalInput").ap()
    bsr_d = nc.dram_tensor("bsr", [1, C], bf16, kind="ExternalInput").ap()
    bsc_d = nc.dram_tensor("bsc", [P, OC], bf16, kind="ExternalInput").ap()
    onec_d = nc.dram_tensor("onec", [P, 1], bf16, kind="ExternalInput").ap()
    oner_d = nc.dram_tensor("oner", [1, P], bf16, kind="ExternalInput").ap()
    gm_d = nc.dram_tensor("gm", [P, OC], f32, kind="ExternalInput").ap()
    bt_d = nc.dram_tensor("bt", [P, OC], f32, kind="ExternalInput").ap()
    out_d = nc.dram_tensor("out", [BPC, C, N], f32, kind="ExternalOutput").ap()
    st_in_d = nc.dram_tensor("stats_in", [P, 2 * OC], f32).ap()
    st_out_d = nc.dram_tensor("stats_out", [P, 2 * OC], f32,
                              addr_space="Shared").ap()
    if debug_dumps:
        bf16_ = dt.float16
        dbg = {
            "dbg_id": nc.dram_tensor("dbg_id", [P, P], bf16_,
                                     kind="ExternalOutput").ap(),
            "dbg_z": nc.dram_tensor("dbg_z", [P, NT, K], bf16_,
                                    kind="ExternalOutput").ap(),
            "dbg_mut": nc.dram_tensor("dbg_mut", [P, OC, K], bf16_,
                                      kind="ExternalOutput").ap(),
            "dbg_muT": nc.dram_tensor("dbg_muT", [K, C], bf16_,
                                      kind="ExternalOutput").ap(),
            "dbg_rs": nc.dram_tensor("dbg_rs", [K, 1], f32,
                                     kind="ExternalOutput").ap(),
            "dbg_h": nc.dram_tensor("dbg_h", [P, OC, N], bf16_,
                                    kind="ExternalOutput").ap(),
            "dbg_pack": nc.dram_tensor("dbg_pack", [P, 2 * OC], f32,
                                       kind="ExternalOutput").ap(),
            "dbg_ab": nc.dram_tensor("dbg_ab", [P, 2 * OC], f32,
                                     kind="ExternalOutput").ap(),
            "dbg_xb": nc.dram_tensor("dbg_xb", [P, OC, 1024], bf16_,
                                     kind="ExternalOutput").ap(),
            "dbg_xt": nc.dram_tensor("dbg_xt", [P, NT, C], bf16_,
                                     kind="ExternalOutput").ap(),
            "dbg_et": nc.dram_tensor("dbg_et", [P, 512], f32,
                                     kind="ExternalOutput").ap(),
            "dbg_mut0": nc.dram_tensor("dbg_mut0", [P, OC, K], bf16_,
                                       kind="ExternalOutput").ap(),
            "dbg_gx": nc.dram_tensor("dbg_gx", [P, OC, K], bf16_,
                                     kind="ExternalOutput").ap(),
            "dbg_sk": nc.dram_tensor("dbg_sk", [1, K], bf16_,
                                     kind="ExternalOutput").ap(),
            "dbg_rsr": nc.dram_tensor("dbg_rsr", [1, K], bf16_,
                                      kind="ExternalOutput").ap(),
            "dbg_mun": nc.dram_tensor("dbg_mun", [P, OC, K], bf16_,
                                      kind="ExternalOutput").ap(),
            "dbg_nsq": nc.dram_tensor("dbg_nsq", [1, K], f32,
                                      kind="ExternalOutput").ap(),
        }

    with tile.TileContext(nc) as tc, ExitStack() as ctx:
        consts = ctx.enter_context(tc.tile_pool(name="consts", bufs=1))
        xbig = ctx.enter_context(tc.tile_pool(name="xbig", bufs=1))
        zpool = ctx.enter_context(tc.tile_pool(name="zpool", bufs=1))
        mutp = ctx.enter_context(tc.tile_pool(name="mutp", bufs=2))
        smalls = ctx.enter_context(tc.tile_pool(name="smalls", bufs=2))
        statp = ctx.enter_context(tc.tile_pool(name="statp", bufs=1))

        # ---- constants ----
        idb = consts.tile([P, P], bf16)
        make_identity(nc, idb[:])
        if debug_dumps:
            nc.sync.dma_start(dbg["dbg_id"], idb[:])
        ws_sb = consts.tile([P, OC, C], bf16)
        nc.sync.dma_start(ws_sb[:], ws_d)
        wst_sb = consts.tile([P, OC, C], bf16)
        nc.sync.dma_start(wst_sb[:], wst_d)
        mut0_sb = consts.tile([P, OC, K], bf16)
        nc.sync.dma_start(mut0_sb[:], mut0_d)
        mub0_sb = consts.tile([1, K], bf16)
        nc.sync.dma_start(mub0_sb[:], mub0_d)
        bsr_sb = consts.tile([1, C], bf16)
        nc.sync.dma_start(bsr_sb[:], bsr_d)
        bsc_sb = consts.tile([P, OC], bf16)
        nc.sync.dma_start(bsc_sb[:], bsc_d)
        onec_sb = consts.tile([P, 1], bf16)
        nc.sync.dma_start(onec_sb[:], onec_d)
        oner_sb = consts.tile([1, P], bf16)
        nc.sync.dma_start(oner_sb[:], oner_d)
        gm_sb = consts.tile([P, OC], f32)
        nc.sync.dma_start(gm_sb[:], gm_d)
        bt_sb = consts.tile([P, OC], f32)
        nc.sync.dma_start(bt_sb[:], bt_d)
        eps_sb = consts.tile([P, 1], f32)
        nc.vector.memset(eps_sb[:], BN_EPS)

        xb_sb = [xbig.tile([P, OC, N], bf16, name=f"xb{b}") for b in range(BPC)]
        z_sb = [zpool.tile([P, NT, K], bf16, name=f"z{b}") for b in range(BPC)]

        sum_acc = statp.tile([P, OC, BPC * (N // 512)], f32)
        sq_acc = statp.tile([P, OC, BPC * (N // 512)], f32)

        mut_cur = [mut0_sb, mut0_sb]
        mub_cur = [mub0_sb, mub0_sb]
        rs_col = [None] * BPC
        muT_sb = [None] * BPC

        # ================= EM phase (both images, interleaved) ============
        with ExitStack() as l2:
            xtp = l2.enter_context(tc.tile_pool(name="xtp", bufs=1))
            etp = l2.enter_context(tc.tile_pool(name="etp", bufs=3))
            psum2 = l2.enter_context(tc.tile_pool(name="psum2", bufs=1,
                                                  space="PSUM"))

            xt_sb = [xtp.tile([P, NT, C], bf16, name=f"xt{b}")
                     for b in range(BPC)]

            # x loads, chunked for pipelining; natural + transposed layouts
            for b in range(BPC):
                for q in range(4):
                    nc.sync.dma_start(
                        xb_sb[b][:, :, q * 1024:(q + 1) * 1024],
                        xb_d[b, :, :, q * 1024:(q + 1) * 1024])
                    nc.sync.dma_start(
                        xt_sb[b][:, q * 8:(q + 1) * 8, :],
                        xt_d[b, :, q * 8:(q + 1) * 8, :])

            def psf(tag, name):
                return psum2.tile([P, 512], f32, tag=tag, name=name)

            def em_iter(b, it):
                mut, mub = mut_cur[b], mub_cur[b]
                g_ps = psf(f"G{b}", f"g{b}_{it}")
                for bank in range(NBK):
                    a_ps = psf("A%d" % (bank % 2), f"a{b}{it}{bank}")
                    for t8 in range(TPB):
                        t = bank * TPB + t8
                        sl = a_ps[:, t8 * K:(t8 + 1) * K]
                        for ct in range(OC):
                            nc.tensor.matmul(
                                sl, xb_sb[b][:, ct, t * P:(t + 1) * P],
                                mut[:, ct, :],
                                start=(ct == 0), stop=False)
                        nc.tensor.matmul(sl, oner_sb[:], mub[:],
                                         start=False, stop=True)
                    et = etp.tile([P, TPB * K], f32, tag="et",
                                  name=f"et{b}{it}{bank}")
                    nc.scalar.activation(et[:], a_ps[:], AF.Exp)
                    if debug_dumps and b == 0 and it == 0 and bank == 0:
                        nc.sync.dma_start(dbg["dbg_et"], et[:])
                    et3 = et[:].rearrange("p (t k) -> p t k", k=K)
                    s8 = smalls.tile([P, TPB], f32, tag="s8", bufs=3)
                    nc.vector.tensor_reduce(s8[:], et3, axis=AX.X, op=ALU.add)
                    nc.vector.reciprocal(s8[:], s8[:])
                    zsl = z_sb[b][:, bank * TPB:(bank + 1) * TPB, :]
                    nc.vector.tensor_tensor(
                        zsl, et3, s8[:, :, None].to_broadcast((P, TPB, K)),
                        ALU.mult)
                # accumulation chains must not interleave: complete each
                # PSUM group before opening the next (PE corrupts otherwise)
                for cj in range(OC):
                    for t in range(NT):
                        nc.tensor.matmul(
                            g_ps[:, cj * K:(cj + 1) * K],
                            xt_sb[b][:, t, cj * P:(cj + 1) * P],
                            z_sb[b][:, t, :],
                            start=(t == 0), stop=(t == NT - 1))
                for t in range(NT):
                    nc.tensor.matmul(g_ps[:1, OC * K:(OC + 1) * K],
                                     onec_sb[:], z_sb[b][:, t, :],
                                     start=(t == 0), stop=(t == NT - 1))
                # ---- mu update tail ----
                gx = smalls.tile([P, OC, K], bf16, tag="gx")
                nc.scalar.copy(gx[:], g_ps[:, :OC * K])
                skr = smalls.tile([1, K], bf16, tag="sk")
                nc.vector.tensor_copy(skr[:], g_ps[:1, OC * K:(OC + 1) * K])
                mur_ps = psf("MU", f"mur{b}{it}")
                for o2 in range(OC):
                    msl = mur_ps[:, o2 * K:(o2 + 1) * K]
                    for ct in range(OC):
                        nc.tensor.matmul(msl,
                                         wst_sb[:, ct, o2 * P:(o2 + 1) * P],
                                         gx[:, ct, :],
                                         start=(ct == 0), stop=False)
                    nc.tensor.matmul(msl, bsr_sb[:, o2 * P:(o2 + 1) * P],
                                     skr[:], start=False, stop=True)
                sq = smalls.tile([P, OC, K], bf16, tag="sq")  # mur^2 <= ~5e3, fp16-safe
                nc.scalar.square(sq[:], mur_ps[:, :OC * K])
                nsl = mur_ps[:1, OC * K:(OC + 1) * K]
                for j in range(OC):
                    nc.tensor.matmul(nsl, onec_sb[:], sq[:, j, :],
                                     start=(j == 0), stop=(j == OC - 1))
                if it < NUM_ITER - 1:
                    nr = smalls.tile([1, K], f32, tag="nr")
                    nc.scalar.activation(nr[:], nsl, AF.Ln)
                    rsr = smalls.tile([1, K], bf16, tag="rsr")
                    nc.scalar.activation(rsr[:], nr[:], AF.Exp, scale=-0.5)
                    rep = mur_ps[:, (OC + 1) * K:(OC + 2) * K]
                    nc.tensor.matmul(rep, oner_sb[:], rsr[:],
                                     start=True, stop=True)
                    rep_sb = smalls.tile([P, K], f32, tag="rep")
                    nc.scalar.copy(rep_sb[:], rep)
                    mun = mutp.tile([P, OC, K], bf16, tag=f"mun{b}",
                                    name=f"mun{b}_{it}")
                    nc.vector.tensor_tensor(
                        mun[:],
                        mur_ps[:, :OC * K].rearrange("p (t k) -> p t k", k=K),
                        rep_sb[:, None, :].to_broadcast((P, OC, K)), ALU.mult)
                    mtn_ps = psf("MU2", f"mtn{b}{it}")
                    for cj in range(OC):
                        msl = mtn_ps[:, cj * K:(cj + 1) * K]
                        for ct in range(OC):
                            nc.tensor.matmul(
                                msl, ws_sb[:, ct, cj * P:(cj + 1) * P],
                                mun[:, ct, :],
                                start=(ct == 0), stop=(ct == OC - 1))
                    mutn = mutp.tile([P, OC, K], bf16, tag=f"mut{b}",
                                     name=f"mut{b}_{it}")
                    nc.scalar.copy(mutn[:], mtn_ps[:, :OC * K])
                    mut_cur[b] = mutn
                    if debug_dumps and b == 0 and it == 0:
                        nc.sync.dma_start(dbg["dbg_gx"], gx[:])
                        nc.sync.dma_start(dbg["dbg_sk"], skr[:])
                        nc.sync.dma_start(dbg["dbg_rsr"], rsr[:])
                        nc.sync.dma_start(dbg["dbg_mun"], mun[:])
                        nsq_sb = smalls.tile([1, K], f32, tag="dbgnsq")
                        nc.vector.tensor_copy(nsq_sb[:], nsl)
                        nc.sync.dma_start(dbg["dbg_nsq"], nsq_sb[:])
                    bsl = mtn_ps[:1, OC * K:(OC + 1) * K]
                    for ct in range(OC):
                        nc.tensor.matmul(bsl, bsc_sb[:, ct:ct + 1],
                                         mun[:, ct, :],
                                         start=(ct == 0), stop=(ct == OC - 1))
                    mubn = smalls.tile([1, K], bf16, tag=f"mub{b}")
                    nc.vector.tensor_copy(mubn[:], bsl)
                    mub_cur[b] = mubn
                else:
                    # rsqrt of col norms as a [K,1] column for the zT scale
                    nr2 = smalls.tile([1, K], f32, tag="nr2")
                    nc.scalar.activation(nr2[:], nsl, AF.Ln)
                    rs_row = smalls.tile([1, K], bf16, tag="rsrow")
                    nc.scalar.activation(rs_row[:], nr2[:], AF.Exp,
                                         scale=-0.5)
                    tc_ps = psum2.tile([P, 512], bf16, tag="TC",
                                       name=f"tc{b}")
                    nc.tensor.transpose(tc_ps[:K, 0:1], rs_row[:],
                                        idb[:1, :1])
                    rcol = smalls.tile([K, 1], f32, tag=f"rs{b}", bufs=1)
                    nc.vector.tensor_copy(rcol[:], tc_ps[:K, 0:1])
                    rs_col[b] = rcol
                    murs = smalls.tile([P, OC, K], bf16, tag="murs")
                    nc.vector.tensor_copy(murs[:], mur_ps[:, :OC * K])
                    for j in range(OC):
                        nc.tensor.transpose(tc_ps[:K, j * P:(j + 1) * P],
                                            murs[:, j, :], idb[:])
                    mt = mutp.tile([K, C], bf16, tag=f"muT{b}", bufs=1,
                                   name=f"muT{b}")
                    nc.scalar.copy(mt[:], tc_ps[:K, :])
                    muT_sb[b] = mt

            if debug_dumps:
                nc.sync.dma_start(dbg["dbg_xb"], xb_sb[0][:, :, :1024])
                nc.sync.dma_start(dbg["dbg_xt"], xt_sb[0][:])
                nc.sync.dma_start(dbg["dbg_mut0"], mut0_sb[:])
            for it in range(NUM_ITER):
                for b in range(BPC):
                    em_iter(b, it)
                    if debug_dumps and b == 0 and it == 0:
                        nc.sync.dma_start(dbg["dbg_z"], z_sb[0][:])
                        nc.sync.dma_start(dbg["dbg_mut"], mut_cur[0][:])
            if debug_dumps:
                nc.sync.dma_start(dbg["dbg_muT"], muT_sb[0][:])
                nc.sync.dma_start(dbg["dbg_rs"], rs_col[0][:])

        # ================= L3: y2 / head / BN / final ====================
        with ExitStack() as l3:
            hbig = l3.enter_context(tc.tile_pool(name="hbig", bufs=1))
            ztp = l3.enter_context(tc.tile_pool(name="ztp", bufs=2))
            ry2p = l3.enter_context(tc.tile_pool(name="ry2p", bufs=2))
            junkp = l3.enter_context(tc.tile_pool(name="junkp", bufs=2))
            fstage = l3.enter_context(tc.tile_pool(name="fstage", bufs=3))
            psum3 = l3.enter_context(tc.tile_pool(name="psum3", bufs=1,
                                                  space="PSUM"))

            hwt_sb = consts.tile([P, OC, C], bf16, name="hwt_sb")
            nc.sync.dma_start(hwt_sb[:], hwt_d)

            h_of = [hbig.tile([P, OC, N], bf16, name=f"h{b}")
                    for b in range(BPC)]

            NCH = N // 512
            for b in range(BPC):
                for ch in range(NCH):
                    zt_ps = psum3.tile([P, 512], bf16, tag="ZT",
                                       name=f"zt{b}{ch}")
                    for j in range(4):
                        nc.tensor.transpose(zt_ps[:K, j * P:(j + 1) * P],
                                            z_sb[b][:, ch * 4 + j, :],
                                            idb[:])
                    zts = ztp.tile([K, 512], bf16, tag="zt")
                    nc.vector.tensor_scalar(zts[:], zt_ps[:K, :],
                                            rs_col[b], None, ALU.mult)
                    ry2 = ry2p.tile([P, OC, 512], bf16, tag="ry2")
                    for ot in range(OC):
                        y2_ps = psum3.tile([P, 512], f32, tag="Y2%d" % (ot % 2),
                                           name=f"y2{b}{ch}{ot}")
                        nc.tensor.matmul(y2_ps[:],
                                         muT_sb[b][:, ot * P:(ot + 1) * P],
                                         zts[:], start=True, stop=True)
                        if ot % 2 == 0:
                            nc.vector.tensor_scalar(ry2[:, ot, :], y2_ps[:],
                                                    0.0, None, ALU.max)
                        else:
                            nc.scalar.activation(ry2[:, ot, :], y2_ps[:],
                                                 AF.Relu)
                    acol = b * NCH + ch
                    for o2 in range(OC):
                        h_ps = psum3.tile([P, 512], f32, tag="H%d" % (o2 % 2),
                                          name=f"h{b}{ch}{o2}")
                        for oc in range(OC):
                            nc.tensor.matmul(
                                h_ps[:], hwt_sb[:, oc, o2 * P:(o2 + 1) * P],
                                ry2[:, oc, :],
                                start=(oc == 0), stop=(oc == OC - 1))
                        dap = h_of[b][:, o2, ch * 512:(ch + 1) * 512]
                        junk = junkp.tile([P, 512], bf16, tag="junk")
                        if o2 % 2 == 0:
                            nc.vector.tensor_scalar(
                                dap, h_ps[:], 0.0, 0.0, ALU.add, ALU.add,
                                accum_out=sum_acc[:, o2, acol:acol + 1])
                            nc.scalar.activation(
                                junk[:], h_ps[:], AF.Square,
                                accum_out=sq_acc[:, o2, acol:acol + 1])
                        else:
                            nc.scalar.activation(
                                dap, h_ps[:], AF.Copy,
                                accum_out=sum_acc[:, o2, acol:acol + 1])
                            nc.vector.scalar_tensor_tensor(
                                junk[:], dap, 1.0, dap,
                                ALU.mult, ALU.mult,
                                accum_out=sq_acc[:, o2, acol:acol + 1])

            if debug_dumps:
                nc.sync.dma_start(dbg["dbg_h"], h_of[0][:])
            # ---- BN stats: aggregate, AllReduce, affine coefficients ----
            pack = statp.tile([P, 2 * OC], f32)
            packv = pack[:].rearrange("p (o two) -> p o two", two=2)
            nc.vector.tensor_reduce(packv[:, :, 0:1], sum_acc[:], axis=AX.X,
                                    op=ALU.add)
            nc.vector.tensor_reduce(packv[:, :, 1:2], sq_acc[:], axis=AX.X,
                                    op=ALU.add)
            nc.sync.dma_start(st_in_d[:], pack[:])
            if use_collective:
                nc.gpsimd.collective_compute(
                    "AllReduce", ALU.add,
                    replica_groups=[list(range(n_devices))],
                    ins=[st_in_d[:]],
                    outs=[st_out_d[:]],
                )
            else:
                nc.sync.dma_start(st_out_d[:], st_in_d[:])
            red = statp.tile([P, 2 * OC], f32)
            nc.sync.dma_start(red[:], st_out_d[:])
            a_sb = statp.tile([P, OC], f32)
            b2_sb = statp.tile([P, OC], f32)
            inv_nb = 1.0 / float(B * N)
            for o2 in range(OC):
                mean = smalls.tile([P, 1], f32, tag="mean")
                nc.vector.tensor_scalar(mean[:], red[:, 2 * o2:2 * o2 + 1],
                                        inv_nb, None, ALU.mult)
                var = smalls.tile([P, 1], f32, tag="var")
                nc.vector.tensor_scalar(var[:],
                                        red[:, 2 * o2 + 1:2 * o2 + 2],
                                        inv_nb, None, ALU.mult)
                msq = smalls.tile([P, 1], f32, tag="msq")
                nc.vector.tensor_tensor(msq[:], mean[:], mean[:], ALU.mult)
                nc.vector.tensor_tensor(var[:], var[:], msq[:], ALU.subtract)
                # rstd = exp(-0.5*ln(var+eps))
                nc.scalar.activation(var[:], var[:], AF.Ln, bias=eps_sb[:])
                nc.scalar.activation(var[:], var[:], AF.Exp, scale=-0.5)
                nc.vector.tensor_tensor(a_sb[:, o2:o2 + 1],
                                        gm_sb[:, o2:o2 + 1], var[:],
                                        ALU.mult)
                nc.vector.tensor_tensor(msq[:], mean[:], a_sb[:, o2:o2 + 1],
                                        ALU.mult)
                nc.vector.tensor_tensor(b2_sb[:, o2:o2 + 1],
                                        bt_sb[:, o2:o2 + 1], msq[:],
                                        ALU.subtract)

            if debug_dumps:
                nc.sync.dma_start(dbg["dbg_pack"], pack[:])
                abp = statp.tile([P, 2 * OC], f32, name="abp")
                nc.vector.tensor_copy(abp[:, :OC], a_sb[:])
                nc.vector.tensor_copy(abp[:, OC:], b2_sb[:])
                nc.sync.dma_start(dbg["dbg_ab"], abp[:])
            # ---- final: out = relu(h*a + b2 + x) ----
            fi = 0
            for b in range(BPC):
                for o2 in range(OC):
                    for fc in range(FCH):
                        hap = h_of[b][:, o2, fc * 1024:(fc + 1) * 1024]
                        xap = xb_sb[b][:, o2, fc * 1024:(fc + 1) * 1024]
                        t1 = fstage.tile([P, 1024], bf16, tag="t1")
                        nc.vector.tensor_scalar(t1[:], hap,
                                                a_sb[:, o2:o2 + 1],
                                                b2_sb[:, o2:o2 + 1],
                                                ALU.mult, ALU.add)
                        t2 = fstage.tile([P, 1024], bf16, tag="t2")
                        nc.vector.tensor_tensor(t2[:], t1[:], xap, ALU.add)
                        otile = fstage.tile([P, 1024], f32, tag="ot")
                        if fi % 2 == 0:
                            nc.scalar.activation(otile[:], t2[:], AF.Relu)
                        else:
                            nc.vector.tensor_scalar(otile[:], t2[:],
                                                    0.0, None, ALU.max)
                        nc.sync.dma_start(
                            out_d[b, o2 * P:(o2 + 1) * P,
                                  fc * 1024:(fc + 1) * 1024], otile[:])
                        fi += 1

    _hoist_extra_waits(nc)
    return nc


_ENGINE_SEM_PREFIX = {
    "EngineType.PE": "PE_",
    "EngineType.Activation": "Activation_",
    "EngineType.DVE": "DVE_",
    "EngineType.Pool": "Pool_",
    "EngineType.SP": "SP_",
}


def _hoist_extra_waits(nc):
    """This walrus build rejects compute-engine instructions carrying more
    than one sync wait. Engine queues are strict FIFO, so (a) an
    instruction waiting on its own engine's semaphore is always already
    satisfied -> drop it; (b) any extra waits can be hoisted onto NoOp
    instructions injected just before, one wait each -- identical
    semantics."""
    import concourse.mybir as mybir
    nid = 0
    for blk in nc.m.functions[0].blocks:
        out = []
        changed = False
        for i in blk.instructions:
            si = getattr(i, "sync_info", None)
            eng = str(getattr(i, "engine", None))
            waits = list(si.on_wait) if si and si.on_wait else []
            if len(waits) > 1 and eng in _ENGINE_SEM_PREFIX:
                selfp = _ENGINE_SEM_PREFIX[eng]
                waits = [w for w in waits if not w.ant_name.startswith(selfp)]
                for w in waits[:-1]:
                    nid += 1
                    out.append(mybir.InstNoOp(
                        name=f"I-waitnop-{nid}",
                        engine=i.engine,
                        sync_info=mybir.SyncInfo(on_wait=[w], on_update=[]),
                        bass_nofuse=True,
                    ))
                i.sync_info = mybir.SyncInfo(
                    on_wait=waits[-1:], on_update=list(si.on_update or []))
                changed = True
            out.append(i)
        if changed:
            blk.instructions = out


def get_nc():
    if "nc" not in _cache:
        _cache["nc"] = _build_nc()
    return _cache["nc"]


def run(inputs_by_core, trace=False):
    from concourse.bass_utils import run_bass_kernel_spmd
    nc = get_nc()
    return run_bass_kernel_spmd(nc, inputs_by_core, list(range(NCORES)),
                                trace=trace)


def make_in_maps(x, mu, stem_w, stem_b, head_w, head_b, bn_gamma, bn_beta):
    bf16 = np.float16

    x = np.ascontiguousarray(np.asarray(x, np.float32)).reshape(B, C, N)
    mu = np.asarray(mu, np.float32)
    stem_w = np.asarray(stem_w, np.float32)
    stem_b = np.asarray(stem_b, np.float32)
    head_w = np.asarray(head_w, np.float32)

    def tile_rows(m):  # (C, F) -> (P, OC, F) with row t*P+p -> [p, t]
        return np.ascontiguousarray(
            m.reshape(OC, P, -1).transpose(1, 0, 2).astype(bf16))

    mut0 = stem_w.T @ mu                    # (C, K)
    mub0 = (mu.T @ stem_b)[None, :]         # (1, K)

    common = {
        "ws": tile_rows(stem_w),
        "wst": tile_rows(stem_w.T),
        "hwt": tile_rows(head_w.T),
        "mut0": tile_rows(mut0),
        "mub0": np.ascontiguousarray(mub0.astype(bf16)),
        "bsr": np.ascontiguousarray(stem_b[None, :].astype(bf16)),
        "bsc": np.ascontiguousarray(
            stem_b.reshape(OC, P).T.astype(bf16)),
        "onec": np.ones((P, 1), bf16),
        "oner": np.ones((1, P), bf16),
        "gm": np.ascontiguousarray(
            np.asarray(bn_gamma, np.float32).reshape(OC, P).T),
        "bt": np.ascontiguousarray(
            np.asarray(bn_beta, np.float32).reshape(OC, P).T),
    }
    maps = []
    for i in range(NCORES):
        xc = x[i * BPC:(i + 1) * BPC]                     # (BPC, C, N)
        xbt = xc.reshape(BPC, OC, P, N).transpose(0, 2, 1, 3)
        xtt = (xc.transpose(0, 2, 1)                      # (BPC, N, C)
               .reshape(BPC, NT, P, C).transpose(0, 2, 1, 3))
        maps.append({
            "xb": np.ascontiguousarray(xbt.astype(bf16)),
            "xt": np.ascontiguousarray(xtt.astype(bf16)),
            **common,
        })
    return maps


def kernel(x, mu, stem_w, stem_b, head_w, head_b, bn_gamma, bn_beta):
    in_maps = make_in_maps(x, mu, stem_w, stem_b, head_w, head_b,
                           bn_gamma, bn_beta)
    res = run(in_maps, trace=False)
    out = np.concatenate([res.results[i]["out"] for i in range(NCORES)],
                         axis=0)
    return out.reshape(B, C, 64, 64).astype(np.float32)


# revision 31
# speedup vs baseline: 1.4230x; 1.0441x over previous
"""EMAttention2d (vq_codebook) Trainium2 kernel, v2.

Data parallel over batch: 16 images -> 8 cores x 2 images. BN batch stats
cross-core reduced with a tiny AllReduce.

Key layout change vs v1: the EM loop works in pixel-partition layout so
softmax needs no transposes, and all big matmuls keep a full 128-row
output partition with bf16 moving operands (1 cycle/row on PE):

  per image (X = x[b] as (C,N), host also supplies X^T; both bf16):
    mu~ = Ws^T mu          (C,K)  [stem folded into codebook; host for it0]
    mub = mu^T bs          (K,)   [host for it0]
    repeat 3x:
      A[n,k]  = X^T mu~ + 1 (x) mub    - 4 chunk matmuls + rank-1 per tile
      E       = exp(A)                 (N,K) bf16, pixel-partition
      z       = E / rowsum(E)          softmax over free dim k
      Gx[c,k] = sum_n X[c,n] z[n,k]    - lhsT = X^T tiles, rhs = z tiles
      sk      = 1^T z
      muR     = Ws Gx + bs (x) sk      (C,K) natural layout
      mu      = muR / ||col||_2        - norm via ones^T muR^2 matmuls
    y2   = mu z^T   (relu) -> head Hw  - z^T via 1c/r bf16 PE transposes
  BN over batch (AllReduce of per-channel sum/sumsq), then
    out = relu(h*a + b2 + x),  a = gamma*rstd, b2 = beta - mean*a

x stays resident in SBUF (bf16) so the final pass reloads nothing.
"""

import sys

for _p in ("/opt/trn_rl_repo",):
    if _p not in sys.path:
        sys.path.insert(0, _p)

import numpy as np

B, C, N, K = 16, 512, 4096, 64
NCORES = 8
BPC = B // NCORES  # images per core
P = 128
OC = C // P   # 4 chunks of channels
NT = N // P   # 32 pixel tiles
NBK = 4       # A-banks per EM iteration
TPB = NT // NBK  # pixel tiles per bank (8)
FCH = N // 1024  # final-pass chunks per (img, o2)
BN_EPS = 1e-5
NUM_ITER = 3

_cache = {}


def _build_nc(n_devices=NCORES, use_collective=True, debug_dumps=False):
    import concourse.bass as bass
    import concourse.mybir as mybir
    import concourse.tile as tile
    from concourse.masks import make_identity
    from contextlib import ExitStack

    dt = mybir.dt
    f32 = dt.float32
    bf16 = dt.float16  # fp16 storage: 8x finer rounding than bf16, same engine rates
    bfr = dt.bfloat16
    AF = mybir.ActivationFunctionType
    ALU = mybir.AluOpType
    AX = mybir.AxisListType

    nc = bass.Bass("TRN2", target_bir_lowering=False, debug=False,
                   num_devices=n_devices)

    xb_d = nc.dram_tensor("xb", [BPC, P, OC, N], bf16, kind="ExternalInput").ap()
    xt_d = nc.dram_tensor("xt", [BPC, P, NT, C], bf16, kind="ExternalInput").ap()
    ws_d = nc.dram_tensor("ws", [P, OC, C], bf16, kind="ExternalInput").ap()
    wst_d = nc.dram_tensor("wst", [P, OC, C], bf16, kind="ExternalInput").ap()
    hwt_d = nc.dram_tensor("hwt", [P, OC, C], bf16, kind="ExternalInput").ap()
    mut0_d = nc.dram_tensor("mut0", [P, OC, K], bf16, kind="ExternalInput").ap()
    mub0_d = nc.dram_tensor("mub0", [1, K], bf16, kind="ExternalInput").ap()
    bsr_d = nc.dram_tensor("bsr", [1, C], bf16, kind="ExternalInput").ap()
    bsc_d = nc.dram_tensor("bsc", [P, OC], bf16, kind="ExternalInput").ap()
    onec_d = nc.dram_tensor("onec", [P, 1], bf16, kind="ExternalInput").ap()
    oner_d = nc.dram_tensor("oner", [1, P], bf16, kind="ExternalInput").ap()
    gm_d = nc.dram_tensor("gm", [P, OC], f32, kind="ExternalInput").ap()
    bt_d = nc.dram_tensor("bt", [P, OC], f32, kind="ExternalInput").ap()
    out_d = nc.dram_tensor("out", [BPC, C, N], bf16, kind="ExternalOutput").ap()
    st_in_d = nc.dram_tensor("stats_in", [P, 2 * OC], f32).ap()
    st_out_d = nc.dram_tensor("stats_out", [P, 2 * OC], f32,
                              addr_space="Shared").ap()
    if debug_dumps:
        bf16_ = dt.float16
        dbg = {
            "dbg_id": nc.dram_tensor("dbg_id", [P, P], bf16_,
                                     kind="ExternalOutput").ap(),
            "dbg_z": nc.dram_tensor("dbg_z", [P, NT, K], bf16_,
                                    kind="ExternalOutput").ap(),
            "dbg_mut": nc.dram_tensor("dbg_mut", [P, OC, K], bf16_,
                                      kind="ExternalOutput").ap(),
            "dbg_muT": nc.dram_tensor("dbg_muT", [K, C], bf16_,
                                      kind="ExternalOutput").ap(),
            "dbg_rs": nc.dram_tensor("dbg_rs", [K, 1], f32,
                                     kind="ExternalOutput").ap(),
            "dbg_h": nc.dram_tensor("dbg_h", [P, OC, N], bf16_,
                                    kind="ExternalOutput").ap(),
            "dbg_pack": nc.dram_tensor("dbg_pack", [P, 2 * OC], f32,
                                       kind="ExternalOutput").ap(),
            "dbg_ab": nc.dram_tensor("dbg_ab", [P, 2 * OC], f32,
                                     kind="ExternalOutput").ap(),
            "dbg_xb": nc.dram_tensor("dbg_xb", [P, OC, 1024], bf16_,
                                     kind="ExternalOutput").ap(),
            "dbg_xt": nc.dram_tensor("dbg_xt", [P, NT, C], bf16_,
                                     kind="ExternalOutput").ap(),
            "dbg_et": nc.dram_tensor("dbg_et", [P, 512], f32,
                                     kind="ExternalOutput").ap(),
            "dbg_mut0": nc.dram_tensor("dbg_mut0", [P, OC, K], bf16_,
                                       kind="ExternalOutput").ap(),
            "dbg_gx": nc.dram_tensor("dbg_gx", [P, OC, K], bf16_,
                                     kind="ExternalOutput").ap(),
            "dbg_sk": nc.dram_tensor("dbg_sk", [1, K], bf16_,
                                     kind="ExternalOutput").ap(),
            "dbg_rsr": nc.dram_tensor("dbg_rsr", [1, K], bf16_,
                                      kind="ExternalOutput").ap(),
            "dbg_mun": nc.dram_tensor("dbg_mun", [P, OC, K], bf16_,
                                      kind="ExternalOutput").ap(),
            "dbg_nsq": nc.dram_tensor("dbg_nsq", [1, K], f32,
                                      kind="ExternalOutput").ap(),
        }

    with tile.TileContext(nc) as tc, ExitStack() as ctx:
        consts = ctx.enter_context(tc.tile_pool(name="consts", bufs=1))
        xbig = ctx.enter_context(tc.tile_pool(name="xbig", bufs=1))
        zpool = ctx.enter_context(tc.tile_pool(name="zpool", bufs=1))
        mutp = ctx.enter_context(tc.tile_pool(name="mutp", bufs=2))
        smalls = ctx.enter_context(tc.tile_pool(name="smalls", bufs=2))
        statp = ctx.enter_context(tc.tile_pool(name="statp", bufs=1))

        # ---- constants ----
        idb = consts.tile([P, P], bf16)
        make_identity(nc, idb[:])
        if debug_dumps:
            nc.sync.dma_start(dbg["dbg_id"], idb[:])
        ws_sb = consts.tile([P, OC, C], bf16)
        nc.sync.dma_start(ws_sb[:], ws_d)
        wst_sb = consts.tile([P, OC, C], bf16)
        nc.sync.dma_start(wst_sb[:], wst_d)
        mut0_sb = consts.tile([P, OC, K], bf16)
        nc.sync.dma_start(mut0_sb[:], mut0_d)
        mub0_sb = consts.tile([1, K], bf16)
        nc.sync.dma_start(mub0_sb[:], mub0_d)
        bsr_sb = consts.tile([1, C], bf16)
        nc.sync.dma_start(bsr_sb[:], bsr_d)
        bsc_sb = consts.tile([P, OC], bf16)
        nc.sync.dma_start(bsc_sb[:], bsc_d)
        onec_sb = consts.tile([P, 1], bf16)
        nc.sync.dma_start(onec_sb[:], onec_d)
        oner_sb = consts.tile([1, P], bf16)
        nc.sync.dma_start(oner_sb[:], oner_d)
        gm_sb = consts.tile([P, OC], f32)
        nc.sync.dma_start(gm_sb[:], gm_d)
        bt_sb = consts.tile([P, OC], f32)
        nc.sync.dma_start(bt_sb[:], bt_d)
        eps_sb = consts.tile([P, 1], f32)
        nc.vector.memset(eps_sb[:], BN_EPS)

        xb_sb = [xbig.tile([P, OC, N], bf16, name=f"xb{b}") for b in range(BPC)]
        z_sb = [zpool.tile([P, NT, K], bf16, name=f"z{b}") for b in range(BPC)]

        sum_acc = statp.tile([P, OC, BPC * (N // 512)], f32)
        sq_acc = statp.tile([P, OC, BPC * (N // 512)], f32)

        mut_cur = [mut0_sb, mut0_sb]
        mub_cur = [mub0_sb, mub0_sb]
        rs_col = [None] * BPC
        muT_sb = [None] * BPC

        # ================= EM phase (both images, interleaved) ============
        with ExitStack() as l2:
            xtp = l2.enter_context(tc.tile_pool(name="xtp", bufs=1))
            etp = l2.enter_context(tc.tile_pool(name="etp", bufs=3))
            psum2 = l2.enter_context(tc.tile_pool(name="psum2", bufs=1,
                                                  space="PSUM"))

            xt_sb = [xtp.tile([P, NT, C], bf16, name=f"xt{b}")
                     for b in range(BPC)]

            # x loads, chunked for pipelining; natural + transposed layouts
            for b in range(BPC):
                for q in range(4):
                    nc.sync.dma_start(
                        xb_sb[b][:, :, q * 1024:(q + 1) * 1024],
                        xb_d[b, :, :, q * 1024:(q + 1) * 1024])
                    nc.sync.dma_start(
                        xt_sb[b][:, q * 8:(q + 1) * 8, :],
                        xt_d[b, :, q * 8:(q + 1) * 8, :])

            def psf(tag, name):
                return psum2.tile([P, 512], f32, tag=tag, name=name)

            def em_iter(b, it):
                mut, mub = mut_cur[b], mub_cur[b]
                g_ps = psf(f"G{b}", f"g{b}_{it}")
                for bank in range(NBK):
                    a_ps = psf("A%d" % (bank % 2), f"a{b}{it}{bank}")
                    for t8 in range(TPB):
                        t = bank * TPB + t8
                        sl = a_ps[:, t8 * K:(t8 + 1) * K]
                        for ct in range(OC):
                            nc.tensor.matmul(
                                sl, xb_sb[b][:, ct, t * P:(t + 1) * P],
                                mut[:, ct, :],
                                start=(ct == 0), stop=False)
                        nc.tensor.matmul(sl, oner_sb[:], mub[:],
                                         start=False, stop=True)
                    et = etp.tile([P, TPB * K], f32, tag="et",
                                  name=f"et{b}{it}{bank}")
                    nc.scalar.activation(et[:], a_ps[:], AF.Exp)
                    if debug_dumps and b == 0 and it == 0 and bank == 0:
                        nc.sync.dma_start(dbg["dbg_et"], et[:])
                    et3 = et[:].rearrange("p (t k) -> p t k", k=K)
                    s8 = smalls.tile([P, TPB], f32, tag="s8", bufs=3)
                    nc.vector.tensor_reduce(s8[:], et3, axis=AX.X, op=ALU.add)
                    nc.vector.reciprocal(s8[:], s8[:])
                    zsl = z_sb[b][:, bank * TPB:(bank + 1) * TPB, :]
                    nc.vector.tensor_tensor(
                        zsl, et3, s8[:, :, None].to_broadcast((P, TPB, K)),
                        ALU.mult)
                # accumulation chains must not interleave: complete each
                # PSUM group before opening the next (PE corrupts otherwise)
                for cj in range(OC):
                    for t in range(NT):
                        nc.tensor.matmul(
                            g_ps[:, cj * K:(cj + 1) * K],
                            xt_sb[b][:, t, cj * P:(cj + 1) * P],
                            z_sb[b][:, t, :],
                            start=(t == 0), stop=(t == NT - 1))
                for t in range(NT):
                    nc.tensor.matmul(g_ps[:1, OC * K:(OC + 1) * K],
                                     onec_sb[:], z_sb[b][:, t, :],
                                     start=(t == 0), stop=(t == NT - 1))
                # ---- mu update tail ----
                gx = smalls.tile([P, OC, K], bf16, tag="gx")
                nc.scalar.copy(gx[:], g_ps[:, :OC * K])
                skr = smalls.tile([1, K], bf16, tag="sk")
                nc.vector.tensor_copy(skr[:], g_ps[:1, OC * K:(OC + 1) * K])
                mur_ps = psf("MU", f"mur{b}{it}")
                for o2 in range(OC):
                    msl = mur_ps[:, o2 * K:(o2 + 1) * K]
                    for ct in range(OC):
                        nc.tensor.matmul(msl,
                                         wst_sb[:, ct, o2 * P:(o2 + 1) * P],
                                         gx[:, ct, :],
                                         start=(ct == 0), stop=False)
                    nc.tensor.matmul(msl, bsr_sb[:, o2 * P:(o2 + 1) * P],
                                     skr[:], start=False, stop=True)
                sq = smalls.tile([P, OC, K], bf16, tag="sq")  # mur^2 <= ~5e3, fp16-safe
                nc.scalar.square(sq[:], mur_ps[:, :OC * K])
                nsl = mur_ps[:1, OC * K:(OC + 1) * K]
                for j in range(OC):
                    nc.tensor.matmul(nsl, onec_sb[:], sq[:, j, :],
                                     start=(j == 0), stop=(j == OC - 1))
                if it < NUM_ITER - 1:
                    nr = smalls.tile([1, K], f32, tag="nr")
                    nc.scalar.activation(nr[:], nsl, AF.Ln)
                    rsr = smalls.tile([1, K], bf16, tag="rsr")
                    nc.scalar.activation(rsr[:], nr[:], AF.Exp, scale=-0.5)
                    rep = mur_ps[:, (OC + 1) * K:(OC + 2) * K]
                    nc.tensor.matmul(rep, oner_sb[:], rsr[:],
                                     start=True, stop=True)
                    rep_sb = smalls.tile([P, K], f32, tag="rep")
                    nc.scalar.copy(rep_sb[:], rep)
                    mun = mutp.tile([P, OC, K], bf16, tag=f"mun{b}",
                                    name=f"mun{b}_{it}")
                    nc.vector.tensor_tensor(
                        mun[:],
                        mur_ps[:, :OC * K].rearrange("p (t k) -> p t k", k=K),
                        rep_sb[:, None, :].to_broadcast((P, OC, K)), ALU.mult)
                    mtn_ps = psf("MU2", f"mtn{b}{it}")
                    for cj in range(OC):
                        msl = mtn_ps[:, cj * K:(cj + 1) * K]
                        for ct in range(OC):
                            nc.tensor.matmul(
                                msl, ws_sb[:, ct, cj * P:(cj + 1) * P],
                                mun[:, ct, :],
                                start=(ct == 0), stop=(ct == OC - 1))
                    mutn = mutp.tile([P, OC, K], bf16, tag=f"mut{b}",
                                     name=f"mut{b}_{it}")
                    nc.scalar.copy(mutn[:], mtn_ps[:, :OC * K])
                    mut_cur[b] = mutn
                    if debug_dumps and b == 0 and it == 0:
                        nc.sync.dma_start(dbg["dbg_gx"], gx[:])
                        nc.sync.dma_start(dbg["dbg_sk"], skr[:])
                        nc.sync.dma_start(dbg["dbg_rsr"], rsr[:])
                        nc.sync.dma_start(dbg["dbg_mun"], mun[:])
                        nsq_sb = smalls.tile([1, K], f32, tag="dbgnsq")
                        nc.vector.tensor_copy(nsq_sb[:], nsl)
                        nc.sync.dma_start(dbg["dbg_nsq"], nsq_sb[:])
                    bsl = mtn_ps[:1, OC * K:(OC + 1) * K]
                    for ct in range(OC):
                        nc.tensor.matmul(bsl, bsc_sb[:, ct:ct + 1],
                                         mun[:, ct, :],
                                         start=(ct == 0), stop=(ct == OC - 1))
                    mubn = smalls.tile([1, K], bf16, tag=f"mub{b}")
                    nc.vector.tensor_copy(mubn[:], bsl)
                    mub_cur[b] = mubn
                else:
                    # rsqrt of col norms as a [K,1] column for the zT scale
                    nr2 = smalls.tile([1, K], f32, tag="nr2")
                    nc.scalar.activation(nr2[:], nsl, AF.Ln)
                    rs_row = smalls.tile([1, K], bf16, tag="rsrow")
                    nc.scalar.activation(rs_row[:], nr2[:], AF.Exp,
                                         scale=-0.5)
                    tc_ps = psum2.tile([P, 512], bf16, tag="TC",
                                       name=f"tc{b}")
                    nc.tensor.transpose(tc_ps[:K, 0:1], rs_row[:],
                                        idb[:1, :1])
                    rcol = smalls.tile([K, 1], f32, tag=f"rs{b}", bufs=1)
                    nc.vector.tensor_copy(rcol[:], tc_ps[:K, 0:1])
                    rs_col[b] = rcol
                    murs = smalls.tile([P, OC, K], bf16, tag="murs")
                    nc.vector.tensor_copy(murs[:], mur_ps[:, :OC * K])
                    for j in range(OC):
                        nc.tensor.transpose(tc_ps[:K, j * P:(j + 1) * P],
                                            murs[:, j, :], idb[:])
                    mt = mutp.tile([K, C], bf16, tag=f"muT{b}", bufs=1,
                                   name=f"muT{b}")
                    nc.scalar.copy(mt[:], tc_ps[:K, :])
                    muT_sb[b] = mt

            if debug_dumps:
                nc.sync.dma_start(dbg["dbg_xb"], xb_sb[0][:, :, :1024])
                nc.sync.dma_start(dbg["dbg_xt"], xt_sb[0][:])
                nc.sync.dma_start(dbg["dbg_mut0"], mut0_sb[:])
            for it in range(NUM_ITER):
                for b in range(BPC):
                    em_iter(b, it)
                    if debug_dumps and b == 0 and it == 0:
                        nc.sync.dma_start(dbg["dbg_z"], z_sb[0][:])
                        nc.sync.dma_start(dbg["dbg_mut"], mut_cur[0][:])
            if debug_dumps:
                nc.sync.dma_start(dbg["dbg_muT"], muT_sb[0][:])
                nc.sync.dma_start(dbg["dbg_rs"], rs_col[0][:])

        # ================= L3: y2 / head / BN / final ====================
        with ExitStack() as l3:
            hbig = l3.enter_context(tc.tile_pool(name="hbig", bufs=1))
            ztp = l3.enter_context(tc.tile_pool(name="ztp", bufs=2))
            ry2p = l3.enter_context(tc.tile_pool(name="ry2p", bufs=2))
            junkp = l3.enter_context(tc.tile_pool(name="junkp", bufs=2))
            fstage = l3.enter_context(tc.tile_pool(name="fstage", bufs=3))
            psum3 = l3.enter_context(tc.tile_pool(name="psum3", bufs=1,
                                                  space="PSUM"))

            hwt_sb = consts.tile([P, OC, C], bf16, name="hwt_sb")
            nc.sync.dma_start(hwt_sb[:], hwt_d)

            h_of = [hbig.tile([P, OC, N], bf16, name=f"h{b}")
                    for b in range(BPC)]

            NCH = N // 512
            for b in range(BPC):
                for ch in range(NCH):
                    zt_ps = psum3.tile([P, 512], bf16, tag="ZT",
                                       name=f"zt{b}{ch}")
                    for j in range(4):
                        nc.tensor.transpose(zt_ps[:K, j * P:(j + 1) * P],
                                            z_sb[b][:, ch * 4 + j, :],
                                            idb[:])
                    zts = ztp.tile([K, 512], bf16, tag="zt")
                    nc.vector.tensor_scalar(zts[:], zt_ps[:K, :],
                                            rs_col[b], None, ALU.mult)
                    ry2 = ry2p.tile([P, OC, 512], bf16, tag="ry2")
                    for ot in range(OC):
                        y2_ps = psum3.tile([P, 512], f32, tag="Y2%d" % (ot % 2),
                                           name=f"y2{b}{ch}{ot}")
                        nc.tensor.matmul(y2_ps[:],
                                         muT_sb[b][:, ot * P:(ot + 1) * P],
                                         zts[:], start=True, stop=True)
                        if (ch * OC + ot) % 4 == 0:
                            nc.vector.tensor_scalar(ry2[:, ot, :], y2_ps[:],
                                                    0.0, None, ALU.max)
                        else:
                            nc.scalar.activation(ry2[:, ot, :], y2_ps[:],
                                                 AF.Relu)
                    acol = b * NCH + ch
                    for o2 in range(OC):
                        h_ps = psum3.tile([P, 512], f32, tag="H%d" % (o2 % 2),
                                          name=f"h{b}{ch}{o2}")
                        for oc in range(OC):
                            nc.tensor.matmul(
                                h_ps[:], hwt_sb[:, oc, o2 * P:(o2 + 1) * P],
                                ry2[:, oc, :],
                                start=(oc == 0), stop=(oc == OC - 1))
                        dap = h_of[b][:, o2, ch * 512:(ch + 1) * 512]
                        junk = junkp.tile([P, 512], bf16, tag="junk")
                        if o2 % 2 == 0:
                            nc.vector.tensor_scalar(
                                dap, h_ps[:], 0.0, 0.0, ALU.add, ALU.add,
                                accum_out=sum_acc[:, o2, acol:acol + 1])
                            nc.scalar.activation(
                                junk[:], h_ps[:], AF.Square,
                                accum_out=sq_acc[:, o2, acol:acol + 1])
                        else:
                            nc.scalar.activation(
                                dap, h_ps[:], AF.Copy,
                                accum_out=sum_acc[:, o2, acol:acol + 1])
                            nc.vector.scalar_tensor_tensor(
                                junk[:], dap, 1.0, dap,
                                ALU.mult, ALU.mult,
                                accum_out=sq_acc[:, o2, acol:acol + 1])

            if debug_dumps:
                nc.sync.dma_start(dbg["dbg_h"], h_of[0][:])
            # ---- BN stats: aggregate, AllReduce, affine coefficients ----
            pack = statp.tile([P, 2 * OC], f32)
            packv = pack[:].rearrange("p (o two) -> p o two", two=2)
            nc.vector.tensor_reduce(packv[:, :, 0:1], sum_acc[:], axis=AX.X,
                                    op=ALU.add)
            nc.vector.tensor_reduce(packv[:, :, 1:2], sq_acc[:], axis=AX.X,
                                    op=ALU.add)
            nc.sync.dma_start(st_in_d[:], pack[:])
            if use_collective:
                nc.gpsimd.collective_compute(
                    "AllReduce", ALU.add,
                    replica_groups=[list(range(n_devices))],
                    ins=[st_in_d[:]],
                    outs=[st_out_d[:]],
                )
            else:
                nc.sync.dma_start(st_out_d[:], st_in_d[:])
            red = statp.tile([P, 2 * OC], f32)
            nc.sync.dma_start(red[:], st_out_d[:])
            a_sb = statp.tile([P, OC], f32)
            b2_sb = statp.tile([P, OC], f32)
            inv_nb = 1.0 / float(B * N)
            for o2 in range(OC):
                mean = smalls.tile([P, 1], f32, tag="mean")
                nc.vector.tensor_scalar(mean[:], red[:, 2 * o2:2 * o2 + 1],
                                        inv_nb, None, ALU.mult)
                var = smalls.tile([P, 1], f32, tag="var")
                nc.vector.tensor_scalar(var[:],
                                        red[:, 2 * o2 + 1:2 * o2 + 2],
                                        inv_nb, None, ALU.mult)
                msq = smalls.tile([P, 1], f32, tag="msq")
                nc.vector.tensor_tensor(msq[:], mean[:], mean[:], ALU.mult)
                nc.vector.tensor_tensor(var[:], var[:], msq[:], ALU.subtract)
                # rstd = exp(-0.5*ln(var+eps))
                nc.scalar.activation(var[:], var[:], AF.Ln, bias=eps_sb[:])
                nc.scalar.activation(var[:], var[:], AF.Exp, scale=-0.5)
                nc.vector.tensor_tensor(a_sb[:, o2:o2 + 1],
                                        gm_sb[:, o2:o2 + 1], var[:],
                                        ALU.mult)
                nc.vector.tensor_tensor(msq[:], mean[:], a_sb[:, o2:o2 + 1],
                                        ALU.mult)
                nc.vector.tensor_tensor(b2_sb[:, o2:o2 + 1],
                                        bt_sb[:, o2:o2 + 1], msq[:],
                                        ALU.subtract)

            if debug_dumps:
                nc.sync.dma_start(dbg["dbg_pack"], pack[:])
                abp = statp.tile([P, 2 * OC], f32, name="abp")
                nc.vector.tensor_copy(abp[:, :OC], a_sb[:])
                nc.vector.tensor_copy(abp[:, OC:], b2_sb[:])
                nc.sync.dma_start(dbg["dbg_ab"], abp[:])
            # ---- final: out = relu(h*a + b2 + x) ----
            fi = 0
            for b in range(BPC):
                for o2 in range(OC):
                    for fc in range(FCH):
                        hap = h_of[b][:, o2, fc * 1024:(fc + 1) * 1024]
                        xap = xb_sb[b][:, o2, fc * 1024:(fc + 1) * 1024]
                        t1 = fstage.tile([P, 1024], bf16, tag="t1")
                        nc.vector.tensor_scalar(t1[:], hap,
                                                a_sb[:, o2:o2 + 1],
                                                b2_sb[:, o2:o2 + 1],
                                                ALU.mult, ALU.add)
                        t2 = fstage.tile([P, 1024], bf16, tag="t2")
                        if fi % 4 == 3:
                            nc.gpsimd.tensor_tensor(t2[:], t1[:], xap,
                                                    ALU.add)
                        else:
                            nc.vector.tensor_tensor(t2[:], t1[:], xap,
                                                    ALU.add)
                        otile = fstage.tile([P, 1024], bf16, tag="ot")
                        if fi % 4 == 0:
                            nc.vector.tensor_scalar(otile[:], t2[:],
                                                    0.0, None, ALU.max)
                        else:
                            nc.scalar.activation(otile[:], t2[:], AF.Relu)
                        nc.sync.dma_start(
                            out_d[b, o2 * P:(o2 + 1) * P,
                                  fc * 1024:(fc + 1) * 1024], otile[:])
                        fi += 1

    _hoist_extra_waits(nc)
    return nc


_ENGINE_SEM_PREFIX = {
    "EngineType.PE": "PE_",
    "EngineType.Activation": "Activation_",
    "EngineType.DVE": "DVE_",
    "EngineType.Pool": "Pool_",
    "EngineType.SP": "SP_",
}


def _hoist_extra_waits(nc):
    """This walrus build rejects compute-engine instructions carrying more
    than one sync wait. Engine queues are strict FIFO, so (a) an
    instruction waiting on its own engine's semaphore is always already
    satisfied -> drop it; (b) any extra waits can be hoisted onto NoOp
    instructions injected just before, one wait each -- identical
    semantics."""
    import concourse.mybir as mybir
    nid = 0
    for blk in nc.m.functions[0].blocks:
        out = []
        changed = False
        for i in blk.instructions:
            si = getattr(i, "sync_info", None)
            eng = str(getattr(i, "engine", None))
            waits = list(si.on_wait) if si and si.on_wait else []
            if len(waits) > 1 and eng in _ENGINE_SEM_PREFIX:
                selfp = _ENGINE_SEM_PREFIX[eng]
                waits = [w for w in waits if not w.ant_name.startswith(selfp)]
                for w in waits[:-1]:
                    nid += 1
                    out.append(mybir.InstNoOp(
                        name=f"I-waitnop-{nid}",
                        engine=i.engine,
                        sync_info=mybir.SyncInfo(on_wait=[w], on_update=[]),
                        bass_nofuse=True,
                    ))
                i.sync_info = mybir.SyncInfo(
                    on_wait=waits[-1:], on_update=list(si.on_update or []))
                changed = True
            out.append(i)
        if changed:
            blk.instructions = out


def get_nc():
    if "nc" not in _cache:
        _cache["nc"] = _build_nc()
    return _cache["nc"]


def run(inputs_by_core, trace=False):
    from concourse.bass_utils import run_bass_kernel_spmd
    nc = get_nc()
    return run_bass_kernel_spmd(nc, inputs_by_core, list(range(NCORES)),
                                trace=trace)


def make_in_maps(x, mu, stem_w, stem_b, head_w, head_b, bn_gamma, bn_beta):
    bf16 = np.float16

    x = np.ascontiguousarray(np.asarray(x, np.float32)).reshape(B, C, N)
    mu = np.asarray(mu, np.float32)
    stem_w = np.asarray(stem_w, np.float32)
    stem_b = np.asarray(stem_b, np.float32)
    head_w = np.asarray(head_w, np.float32)

    def tile_rows(m):  # (C, F) -> (P, OC, F) with row t*P+p -> [p, t]
        return np.ascontiguousarray(
            m.reshape(OC, P, -1).transpose(1, 0, 2).astype(bf16))

    mut0 = stem_w.T @ mu                    # (C, K)
    mub0 = (mu.T @ stem_b)[None, :]         # (1, K)

    common = {
        "ws": tile_rows(stem_w),
        "wst": tile_rows(stem_w.T),
        "hwt": tile_rows(head_w.T),
        "mut0": tile_rows(mut0),
        "mub0": np.ascontiguousarray(mub0.astype(bf16)),
        "bsr": np.ascontiguousarray(stem_b[None, :].astype(bf16)),
        "bsc": np.ascontiguousarray(
            stem_b.reshape(OC, P).T.astype(bf16)),
        "onec": np.ones((P, 1), bf16),
        "oner": np.ones((1, P), bf16),
        "gm": np.ascontiguousarray(
            np.asarray(bn_gamma, np.float32).reshape(OC, P).T),
        "bt": np.ascontiguousarray(
            np.asarray(bn_beta, np.float32).reshape(OC, P).T),
    }
    maps = []
    for i in range(NCORES):
        xc = x[i * BPC:(i + 1) * BPC]                     # (BPC, C, N)
        xbt = xc.reshape(BPC, OC, P, N).transpose(0, 2, 1, 3)
        xtt = (xc.transpose(0, 2, 1)                      # (BPC, N, C)
               .reshape(BPC, NT, P, C).transpose(0, 2, 1, 3))
        maps.append({
            "xb": np.ascontiguousarray(xbt.astype(bf16)),
            "xt": np.ascontiguousarray(xtt.astype(bf16)),
            **common,
        })
    return maps


def kernel(x, mu, stem_w, stem_b, head_w, head_b, bn_gamma, bn_beta):
    in_maps = make_in_maps(x, mu, stem_w, stem_b, head_w, head_b,
                           bn_gamma, bn_beta)
    res = run(in_maps, trace=False)
    out = np.concatenate([res.results[i]["out"] for i in range(NCORES)],
                         axis=0)
    return out.reshape(B, C, 64, 64).astype(np.float32)
